# revision 47
# baseline (speedup 1.0000x reference)
"""Paged-attention decode kernel for one TRN2 chip (8 NeuronCores).

Problem: B=16 decode sequences, H=8 kv-heads, D=128 head_dim, paged KV cache
(2048 blocks x 16 tokens), T=2048 tokens/sequence, q_per_seq=1.

Strategy (memory-bound regime):
  - Host: scatter new k/v into cache copies at slot_mapping, gather each
    sequence's tokens (block_tables), zero tokens >= context_len, cast to
    bf16, pre-transpose K to [d, t] layout.
  - Shard 2 sequences per core, embarrassingly parallel (no collectives).
  - Device, per (seq, 128-token tile): 8 matmuls with the K^T tile as the
    stationary operand and q_h as the moving operand produce scores already
    transposed ([t, h] in PSUM, partition 0 aligned). ScalarE exp -> p (bf16).
    A matmul against a ones column accumulates the softmax denominator [H, 1];
    two M=8 matmuls p^T V accumulate the numerator [H, 512] x2 in PSUM (only
    the diagonal blocks are used; the host extracts them after normalization).
    Validity masking is free: invalid K columns are zeroed on the host so
    exp(score)=1 there, corrected by subtracting the invalid count from the
    denominator; invalid V rows are zeroed.
"""

import sys

sys.path.insert(0, "/opt/trn_rl_repo")

import contextlib
import os

import ml_dtypes
import numpy as np

import concourse.bass as bass  # noqa: F401  (bass must import before bacc)
import concourse.mybir as mybir
import concourse.tile as tile
from concourse import bacc
from concourse.bass import ts
from concourse.bass_utils import run_bass_kernel_spmd

B, H, D = 16, 8, 128
BS, BPS = 16, 128
NBLK = B * BPS
T = BPS * BS  # 2048 tokens per sequence
NCORES = 8
SPC = B // NCORES  # sequences per core = 2
NT = T // 128  # 128-token tiles per sequence = 16
SCALE = 0.08838834764831845

BF16 = mybir.dt.bfloat16
F32 = mybir.dt.float32
NP_BF16 = ml_dtypes.bfloat16

# DMA chunking: K and V both arrive in 4 chunks per sequence (1 MiB each),
# interleaved in consumption order so compute starts after the first chunk.
NG = 4  # chunks per sequence (512 tokens each)
VG = NT // NG  # 128-token tiles per V dma group = 4


def _kernel_body(tc, kT, v, q, ninv, out):
    nc = tc.nc
    with contextlib.ExitStack() as ctx:
        kp = ctx.enter_context(tc.tile_pool(name="kp", bufs=8))
        vp = ctx.enter_context(tc.tile_pool(name="vp", bufs=10))
        cst = ctx.enter_context(tc.tile_pool(name="cst", bufs=1))
        etp = ctx.enter_context(tc.tile_pool(name="etp", bufs=8))
        zp = ctx.enter_context(tc.tile_pool(name="zp", bufs=2))
        op = ctx.enter_context(tc.tile_pool(name="op", bufs=2))
        ps_s = ctx.enter_context(tc.tile_pool(name="ps_s", bufs=4, space="PSUM"))
        ps_z = ctx.enter_context(tc.tile_pool(name="ps_z", bufs=2, space="PSUM"))
        ps_o = ctx.enter_context(tc.tile_pool(name="ps_o", bufs=1, space="PSUM"))

        qsb = cst.tile([128, SPC * H], BF16, tag="q")
        nc.gpsimd.dma_start(qsb[:], q[:])
        ninv_sb = cst.tile([H, SPC], F32, tag="ninv")
        nc.gpsimd.dma_start(ninv_sb[:], ninv[:])
        ones = cst.tile([128, 1], BF16, tag="ones")
        nc.gpsimd.memset(ones[:], 1.0)
        zb = cst.tile([128, 1], F32, tag="zb")
        nc.gpsimd.memset(zb[:], 0.0)

        # stream K^T and V chunks on the sync HWDGE ring in strict consumption
        # order (k0,v0,k1,v1,...): FIFO arrival matches compute order, and no
        # compute engine's sequencer is blocked by descriptor generation
        kt = {}
        vt = {}
        for s in range(SPC):
            for g in range(NG):
                ktile = kp.tile([128, H, T // NG], BF16, tag="kt", name=f"kt{s}_{g}")
                nc.sync.dma_start(ktile[:], kT[s, g])
                kt[s, g] = ktile
                vtile = vp.tile(
                    [128, VG, H * D], BF16, tag="vt", name=f"vt{s}_{g}"
                )
                nc.sync.dma_start(vtile[:], v[s, g])
                vt[s, g] = vtile

        for s in range(SPC):
            zps = ps_z.tile([H, 1], F32, tag="zps")
            pso = []
            for half in range(2):
                pso_half = ps_o.tile(
                    [H, 512], F32, tag=f"pso{half}", name=f"pso{half}_{s}"
                )
                pso.append(pso_half)
            for i in range(NT):
                # scores^T for 128 tokens x 8 heads, partition-0 aligned
                sT = ps_s.tile([128, H], F32, tag="sT")
                for h in range(H):
                    nc.tensor.matmul(
                        sT[:, h : h + 1],
                        kt[s, i // VG][:, h, ts(i % VG, 128)],
                        qsb[:, s * H + h : s * H + h + 1],
                        start=True,
                        stop=True,
                    )
                et = etp.tile([128, H], BF16, tag="et")
                nc.scalar.activation(
                    et[:],
                    sT[:],
                    mybir.ActivationFunctionType.Exp,
                    bias=zb[:],
                )
                # denominator: z[h] += sum_t exp[t, h]
                nc.tensor.matmul(
                    zps[:],
                    et[:],
                    ones[:],
                    start=(i == 0),
                    stop=(i == NT - 1),
                )
                # numerator: pso[half][m, (h', d)] += sum_t exp[t, m] V[t, h', d]
                # (only the diagonal blocks m == h' are used)
                vtile_n = vt[s, i // VG][:, i % VG, :]
                for half in range(2):
                    nc.tensor.matmul(
                        pso[half][:],
                        et[:],
                        vtile_n[:, ts(half, 512)],
                        start=(i == 0),
                        stop=(i == NT - 1),
                    )

            # ---- normalize (row h by 1/Z_h; junk columns scaled too) ----
            zsb = zp.tile([H, 1], F32, tag="zsb")
            nc.vector.tensor_sub(zsb[:], zps[:], ninv_sb[:, s : s + 1])
            rz = zp.tile([H, 1], F32, tag="rz")
            nc.vector.reciprocal(rz[:], zsb[:])
            osb = op.tile([H, 2, 512], F32, tag="osb")
            for half in range(2):
                nc.vector.tensor_scalar_mul(osb[:, half, :], pso[half][:], rz[:])
            nc.gpsimd.dma_start(out[s], osb[:])


_GRAPH_CACHE = {}


def _install_ntff_hook():
    """Register the axon NTFF profile hook (the image's antenv lacks it)."""
    import sys as _sys

    if "antenv.axon_hooks" in _sys.modules:
        return
    import contextlib as _ctx
    import ctypes
    import types

    so_path = "/opt/axon/libaxon_pjrt.so"
    try:
        lib = ctypes.CDLL(so_path)
    except OSError:
        return
    if not hasattr(lib, "axon_start_nrt_profile"):
        return
    lib.axon_start_nrt_profile.argtypes = [
        ctypes.POINTER(ctypes.c_int64),
        ctypes.c_size_t,
    ]
    lib.axon_start_nrt_profile.restype = ctypes.c_int64
    lib.axon_stop_nrt_profile.argtypes = [ctypes.c_char_p]
    lib.axon_stop_nrt_profile.restype = ctypes.c_int64

    @_ctx.contextmanager
    def _hook(output_dir, device_ids):
        import jax

        jax.devices()
        if device_ids:
            ids = (ctypes.c_int64 * len(device_ids))(*device_ids)
            rc = lib.axon_start_nrt_profile(ids, len(device_ids))
        else:
            rc = lib.axon_start_nrt_profile(None, 0)
        if rc != 0:
            raise RuntimeError(f"axon_start_nrt_profile rc={rc}")
        try:
            yield
        finally:
            n = lib.axon_stop_nrt_profile(str(output_dir).encode())
            print(f"ntff profile: {n} file(s) written to {output_dir}")

    mod = types.ModuleType("antenv.axon_hooks")
    mod.get_axon_ntff_profile_hook = lambda: _hook
    mod.set_axon_ntff_profile_hook = lambda h: None
    _sys.modules["antenv.axon_hooks"] = mod


def _build_graph():
    if "nc" in _GRAPH_CACHE:
        return _GRAPH_CACHE["nc"]
    nc = bacc.Bacc(
        "TRN2", target_bir_lowering=False, debug=False, num_devices=NCORES
    )
    kT = nc.dram_tensor(
        "kT", [SPC, NG, 128, H, T // NG], BF16, kind="ExternalInput"
    ).ap()
    v = nc.dram_tensor(
        "v", [SPC, NG, 128, VG, H * D], BF16, kind="ExternalInput"
    ).ap()
    q = nc.dram_tensor("q", [128, SPC * H], BF16, kind="ExternalInput").ap()
    ninv = nc.dram_tensor("ninv", [H, SPC], F32, kind="ExternalInput").ap()
    # [s, h, 1024]: row h's useful block is cols h*128:(h+1)*128 (host extracts)
    out = nc.dram_tensor("out", [SPC, H, H * D], F32, kind="ExternalOutput").ap()

    with tile.TileContext(nc) as tc:
        _kernel_body(tc, kT, v, q, ninv, out)
    nc.compile()
    _GRAPH_CACHE["nc"] = nc
    return nc


def _prep_shards(q, k, v, k_cache, v_cache, slot_mapping, block_tables, context_lens):
    q = np.asarray(q, np.float32)
    k = np.asarray(k, np.float32)
    v = np.asarray(v, np.float32)
    kc = np.asarray(k_cache, np.float32).reshape(NBLK * BS, H, D).copy()
    vc = np.asarray(v_cache, np.float32).reshape(NBLK * BS, H, D).copy()
    slot = np.asarray(slot_mapping, np.int64)
    bt = np.asarray(block_tables, np.int64)
    ctx = np.asarray(context_lens, np.int64)

    # scatter the newest token's k/v into the caches
    kc[slot] = k
    vc[slot] = v

    # gather each sequence's tokens (fast path: arange block tables)
    if np.array_equal(bt.ravel(), np.arange(B * BPS)):
        k_seq = kc.reshape(B, T, H, D)
        v_seq = vc.reshape(B, T, H, D)
    else:
        tok = bt[:, np.arange(T) // BS] * BS + (np.arange(T) % BS)  # [B, T]
        k_seq = kc[tok]
        v_seq = vc[tok]

    q_scaled = (q * SCALE).astype(NP_BF16)

    in_maps = []
    for c in range(NCORES):
        s0 = c * SPC
        ks = k_seq[s0 : s0 + SPC].astype(NP_BF16)  # [SPC, T, H, D]
        vs = v_seq[s0 : s0 + SPC].astype(NP_BF16)
        for s in range(SPC):
            cl = int(ctx[s0 + s])
            if cl < T:
                ks[s, cl:] = 0
                vs[s, cl:] = 0
        # K^T: [SPC, T, H, D] -> [SPC, H, D=128, NG, T/NG] -> [SPC, NG, 128, H, T/NG]
        ksT = ks.transpose(0, 2, 3, 1).reshape(SPC, H, 128, NG, T // NG)
        ksT = np.ascontiguousarray(ksT.transpose(0, 3, 2, 1, 4))
        # V: [SPC, T, H*D] -> [SPC, NG, VG, 128, H*D] -> [SPC, NG, 128, VG, H*D]
        vsr = vs.reshape(SPC, NG, VG, 128, H * D).transpose(0, 1, 3, 2, 4)
        vsr = np.ascontiguousarray(vsr)
        # q: [SPC, H, D] -> [128, SPC*H]
        qs = np.ascontiguousarray(
            q_scaled[s0 : s0 + SPC].transpose(2, 0, 1).reshape(128, SPC * H)
        )
        # invalid-token count per (h, s): broadcast over heads
        ninv = np.tile(
            (T - np.minimum(ctx[s0 : s0 + SPC], T)).astype(np.float32), (H, 1)
        )
        in_maps.append({"kT": ksT, "v": vsr, "q": qs, "ninv": ninv})
    return in_maps


def kernel(q, k, v, k_cache, v_cache, slot_mapping, block_tables, context_lens):
    in_maps = _prep_shards(
        q, k, v, k_cache, v_cache, slot_mapping, block_tables, context_lens
    )
    nc = _build_graph()
    trace = bool(int(os.environ.get("BASSKV_TRACE", "0")))
    if trace:
        _install_ntff_hook()
    res = run_bass_kernel_spmd(
        nc, in_maps, core_ids=list(range(NCORES)), trace=trace
    )
    if trace:
        _GRAPH_CACHE["last_result"] = res
        print(f"HW exec time: {res.exec_time_ns} ns")
    full = np.stack(
        [np.asarray(res.results[c]["out"], np.float32) for c in range(NCORES)]
    ).reshape(B, H, H * D)
    # extract each head's diagonal block
    out = np.empty((B, H, D), np.float32)
    for h in range(H):
        out[:, h, :] = full[:, h, h * D : (h + 1) * D]
    return out


if __name__ == "__main__":
    # smoke test with random inputs
    rng = np.random.default_rng(0)
    inputs = {
        "q": rng.standard_normal((B, H, D), dtype=np.float32),
        "k": rng.standard_normal((B, H, D), dtype=np.float32),
        "v": rng.standard_normal((B, H, D), dtype=np.float32),
        "k_cache": rng.standard_normal((NBLK, BS, H, D), dtype=np.float32),
        "v_cache": rng.standard_normal((NBLK, BS, H, D), dtype=np.float32),
        "slot_mapping": np.arange(B, dtype=np.int32) * T + (T - 1),
        "block_tables": np.arange(B * BPS, dtype=np.int32).reshape(B, BPS),
        "context_lens": np.full((B,), T, dtype=np.int32),
    }
    print(kernel(**inputs).shape)


# revision 48
# speedup vs baseline: 1.0365x; 1.0365x over previous
"""Paged-attention decode kernel for one TRN2 chip (8 NeuronCores).

Problem: B=16 decode sequences, H=8 kv-heads, D=128 head_dim, paged KV cache
(2048 blocks x 16 tokens), T=2048 tokens/sequence, q_per_seq=1.

Strategy (memory-bound regime):
  - Host: scatter new k/v into cache copies at slot_mapping, gather each
    sequence's tokens (block_tables), zero tokens >= context_len, cast to
    bf16, pre-transpose K to [d, t] layout.
  - Shard 2 sequences per core, embarrassingly parallel (no collectives).
  - Device, per (seq, 128-token tile): 8 matmuls with the K^T tile as the
    stationary operand and q_h as the moving operand produce scores already
    transposed ([t, h] in PSUM, partition 0 aligned). ScalarE exp -> p (bf16).
    A matmul against a ones column accumulates the softmax denominator [H, 1];
    two M=8 matmuls p^T V accumulate the numerator [H, 512] x2 in PSUM (only
    the diagonal blocks are used; the host extracts them after normalization).
    Validity masking is free: invalid K columns are zeroed on the host so
    exp(score)=1 there, corrected by subtracting the invalid count from the
    denominator; invalid V rows are zeroed.
"""

import sys

sys.path.insert(0, "/opt/trn_rl_repo")

import contextlib
import os

import ml_dtypes
import numpy as np

import concourse.bass as bass  # noqa: F401  (bass must import before bacc)
import concourse.mybir as mybir
import concourse.tile as tile
from concourse import bacc
from concourse.bass import ts
from concourse.bass_utils import run_bass_kernel_spmd

B, H, D = 16, 8, 128
BS, BPS = 16, 128
NBLK = B * BPS
T = BPS * BS  # 2048 tokens per sequence
NCORES = 8
SPC = B // NCORES  # sequences per core = 2
NT = T // 128  # 128-token tiles per sequence = 16
SCALE = 0.08838834764831845

BF16 = mybir.dt.bfloat16
F32 = mybir.dt.float32
NP_BF16 = ml_dtypes.bfloat16

# DMA chunking: K and V both arrive in 4 chunks per sequence (1 MiB each),
# interleaved in consumption order so compute starts after the first chunk.
NG = 4  # chunks per sequence (512 tokens each)
VG = NT // NG  # 128-token tiles per V dma group = 4


def _kernel_body(tc, kT, v, v_last, q, ninv, out):
    nc = tc.nc
    with contextlib.ExitStack() as ctx:
        kp = ctx.enter_context(tc.tile_pool(name="kp", bufs=8))
        vp = ctx.enter_context(tc.tile_pool(name="vp", bufs=10))
        cst = ctx.enter_context(tc.tile_pool(name="cst", bufs=1))
        etp = ctx.enter_context(tc.tile_pool(name="etp", bufs=8))
        zp = ctx.enter_context(tc.tile_pool(name="zp", bufs=2))
        op = ctx.enter_context(tc.tile_pool(name="op", bufs=2))
        ps_s = ctx.enter_context(tc.tile_pool(name="ps_s", bufs=4, space="PSUM"))
        ps_z = ctx.enter_context(tc.tile_pool(name="ps_z", bufs=2, space="PSUM"))
        ps_o = ctx.enter_context(tc.tile_pool(name="ps_o", bufs=1, space="PSUM"))

        qsb = cst.tile([128, SPC * H], BF16, tag="q")
        nc.gpsimd.dma_start(qsb[:], q[:])
        ninv_sb = cst.tile([H, SPC], F32, tag="ninv")
        nc.gpsimd.dma_start(ninv_sb[:], ninv[:])
        ones = cst.tile([128, 1], BF16, tag="ones")
        nc.gpsimd.memset(ones[:], 1.0)
        zb = cst.tile([128, 1], F32, tag="zb")
        nc.gpsimd.memset(zb[:], 0.0)

        # stream K^T and V chunks on the sync HWDGE ring in strict consumption
        # order (k0,v0,k1,v1,...): FIFO arrival matches compute order, and no
        # compute engine's sequencer is blocked by descriptor generation
        kt = {}
        vt = {}
        for s in range(SPC):
            for g in range(NG):
                ktile = kp.tile([128, H, T // NG], BF16, tag="kt", name=f"kt{s}_{g}")
                nc.sync.dma_start(ktile[:], kT[s, g])
                kt[s, g] = ktile
                if g < NG - 1:
                    vtile = vp.tile(
                        [128, VG, H * D], BF16, tag="vt", name=f"vt{s}_{g}"
                    )
                    nc.sync.dma_start(vtile[:], v[s, g])
                    vt[s, g] = vtile
                else:
                    # last chunk arrives per-tile (contiguous blocks) so the
                    # final PV matmuls start as soon as each 256 KiB lands
                    for n in range(VG):
                        vtile = vp.tile(
                            [128, H * D], BF16, tag="vt4", name=f"vt{s}_{g}_{n}"
                        )
                        nc.sync.dma_start(vtile[:], v_last[s, n])
                        vt[s, g, n] = vtile

        for s in range(SPC):
            zps = ps_z.tile([H, 1], F32, tag="zps")
            pso = []
            for half in range(2):
                pso_half = ps_o.tile(
                    [H, 512], F32, tag=f"pso{half}", name=f"pso{half}_{s}"
                )
                pso.append(pso_half)
            for i in range(NT):
                # scores^T for 128 tokens x 8 heads, partition-0 aligned
                sT = ps_s.tile([128, H], F32, tag="sT")
                for h in range(H):
                    nc.tensor.matmul(
                        sT[:, h : h + 1],
                        kt[s, i // VG][:, h, ts(i % VG, 128)],
                        qsb[:, s * H + h : s * H + h + 1],
                        start=True,
                        stop=True,
                    )
                et = etp.tile([128, H], BF16, tag="et")
                nc.scalar.activation(
                    et[:],
                    sT[:],
                    mybir.ActivationFunctionType.Exp,
                    bias=zb[:],
                )
                # denominator: z[h] += sum_t exp[t, h]
                nc.tensor.matmul(
                    zps[:],
                    et[:],
                    ones[:],
                    start=(i == 0),
                    stop=(i == NT - 1),
                )
                # numerator: pso[half][m, (h', d)] += sum_t exp[t, m] V[t, h', d]
                # (only the diagonal blocks m == h' are used)
                g, n = i // VG, i % VG
                vtile_n = vt[s, g][:, n, :] if g < NG - 1 else vt[s, g, n][:]
                for half in range(2):
                    nc.tensor.matmul(
                        pso[half][:],
                        et[:],
                        vtile_n[:, ts(half, 512)],
                        start=(i == 0),
                        stop=(i == NT - 1),
                    )

            # ---- normalize (row h by 1/Z_h; junk columns scaled too) ----
            zsb = zp.tile([H, 1], F32, tag="zsb")
            nc.vector.tensor_sub(zsb[:], zps[:], ninv_sb[:, s : s + 1])
            rz = zp.tile([H, 1], F32, tag="rz")
            nc.vector.reciprocal(rz[:], zsb[:])
            osb = op.tile([H, 2, 512], F32, tag="osb")
            for half in range(2):
                nc.vector.tensor_scalar_mul(osb[:, half, :], pso[half][:], rz[:])
            nc.sync.dma_start(out[s], osb[:])


_GRAPH_CACHE = {}


def _install_ntff_hook():
    """Register the axon NTFF profile hook (the image's antenv lacks it)."""
    import sys as _sys

    if "antenv.axon_hooks" in _sys.modules:
        return
    import contextlib as _ctx
    import ctypes
    import types

    so_path = "/opt/axon/libaxon_pjrt.so"
    try:
        lib = ctypes.CDLL(so_path)
    except OSError:
        return
    if not hasattr(lib, "axon_start_nrt_profile"):
        return
    lib.axon_start_nrt_profile.argtypes = [
        ctypes.POINTER(ctypes.c_int64),
        ctypes.c_size_t,
    ]
    lib.axon_start_nrt_profile.restype = ctypes.c_int64
    lib.axon_stop_nrt_profile.argtypes = [ctypes.c_char_p]
    lib.axon_stop_nrt_profile.restype = ctypes.c_int64

    @_ctx.contextmanager
    def _hook(output_dir, device_ids):
        import jax

        jax.devices()
        if device_ids:
            ids = (ctypes.c_int64 * len(device_ids))(*device_ids)
            rc = lib.axon_start_nrt_profile(ids, len(device_ids))
        else:
            rc = lib.axon_start_nrt_profile(None, 0)
        if rc != 0:
            raise RuntimeError(f"axon_start_nrt_profile rc={rc}")
        try:
            yield
        finally:
            n = lib.axon_stop_nrt_profile(str(output_dir).encode())
            print(f"ntff profile: {n} file(s) written to {output_dir}")

    mod = types.ModuleType("antenv.axon_hooks")
    mod.get_axon_ntff_profile_hook = lambda: _hook
    mod.set_axon_ntff_profile_hook = lambda h: None
    _sys.modules["antenv.axon_hooks"] = mod


def _build_graph():
    if "nc" in _GRAPH_CACHE:
        return _GRAPH_CACHE["nc"]
    nc = bacc.Bacc(
        "TRN2", target_bir_lowering=False, debug=False, num_devices=NCORES
    )
    kT = nc.dram_tensor(
        "kT", [SPC, NG, 128, H, T // NG], BF16, kind="ExternalInput"
    ).ap()
    v = nc.dram_tensor(
        "v", [SPC, NG - 1, 128, VG, H * D], BF16, kind="ExternalInput"
    ).ap()
    v_last = nc.dram_tensor(
        "v_last", [SPC, VG, 128, H * D], BF16, kind="ExternalInput"
    ).ap()
    q = nc.dram_tensor("q", [128, SPC * H], BF16, kind="ExternalInput").ap()
    ninv = nc.dram_tensor("ninv", [H, SPC], F32, kind="ExternalInput").ap()
    # [s, h, 1024]: row h's useful block is cols h*128:(h+1)*128 (host extracts)
    out = nc.dram_tensor("out", [SPC, H, H * D], F32, kind="ExternalOutput").ap()

    with tile.TileContext(nc) as tc:
        _kernel_body(tc, kT, v, v_last, q, ninv, out)
    nc.compile()
    _GRAPH_CACHE["nc"] = nc
    return nc


def _prep_shards(q, k, v, k_cache, v_cache, slot_mapping, block_tables, context_lens):
    q = np.asarray(q, np.float32)
    k = np.asarray(k, np.float32)
    v = np.asarray(v, np.float32)
    kc = np.asarray(k_cache, np.float32).reshape(NBLK * BS, H, D).copy()
    vc = np.asarray(v_cache, np.float32).reshape(NBLK * BS, H, D).copy()
    slot = np.asarray(slot_mapping, np.int64)
    bt = np.asarray(block_tables, np.int64)
    ctx = np.asarray(context_lens, np.int64)

    # scatter the newest token's k/v into the caches
    kc[slot] = k
    vc[slot] = v

    # gather each sequence's tokens (fast path: arange block tables)
    if np.array_equal(bt.ravel(), np.arange(B * BPS)):
        k_seq = kc.reshape(B, T, H, D)
        v_seq = vc.reshape(B, T, H, D)
    else:
        tok = bt[:, np.arange(T) // BS] * BS + (np.arange(T) % BS)  # [B, T]
        k_seq = kc[tok]
        v_seq = vc[tok]

    q_scaled = (q * SCALE).astype(NP_BF16)

    in_maps = []
    for c in range(NCORES):
        s0 = c * SPC
        ks = k_seq[s0 : s0 + SPC].astype(NP_BF16)  # [SPC, T, H, D]
        vs = v_seq[s0 : s0 + SPC].astype(NP_BF16)
        for s in range(SPC):
            cl = int(ctx[s0 + s])
            if cl < T:
                ks[s, cl:] = 0
                vs[s, cl:] = 0
        # K^T: [SPC, T, H, D] -> [SPC, H, D=128, NG, T/NG] -> [SPC, NG, 128, H, T/NG]
        ksT = ks.transpose(0, 2, 3, 1).reshape(SPC, H, 128, NG, T // NG)
        ksT = np.ascontiguousarray(ksT.transpose(0, 3, 2, 1, 4))
        # V: [SPC, T, H*D] -> [SPC, NG, VG, 128, H*D] -> [SPC, NG, 128, VG, H*D]
        vsg = vs.reshape(SPC, NG, VG, 128, H * D)
        vsr = np.ascontiguousarray(vsg[:, : NG - 1].transpose(0, 1, 3, 2, 4))
        vlast = np.ascontiguousarray(vsg[:, NG - 1])  # [SPC, VG, 128, H*D]
        # q: [SPC, H, D] -> [128, SPC*H]
        qs = np.ascontiguousarray(
            q_scaled[s0 : s0 + SPC].transpose(2, 0, 1).reshape(128, SPC * H)
        )
        # invalid-token count per (h, s): broadcast over heads
        ninv = np.tile(
            (T - np.minimum(ctx[s0 : s0 + SPC], T)).astype(np.float32), (H, 1)
        )
        in_maps.append(
            {"kT": ksT, "v": vsr, "v_last": vlast, "q": qs, "ninv": ninv}
        )
    return in_maps


def kernel(q, k, v, k_cache, v_cache, slot_mapping, block_tables, context_lens):
    in_maps = _prep_shards(
        q, k, v, k_cache, v_cache, slot_mapping, block_tables, context_lens
    )
    nc = _build_graph()
    trace = bool(int(os.environ.get("BASSKV_TRACE", "0")))
    if trace:
        _install_ntff_hook()
    res = run_bass_kernel_spmd(
        nc, in_maps, core_ids=list(range(NCORES)), trace=trace
    )
    if trace:
        _GRAPH_CACHE["last_result"] = res
        print(f"HW exec time: {res.exec_time_ns} ns")
    full = np.stack(
        [np.asarray(res.results[c]["out"], np.float32) for c in range(NCORES)]
    ).reshape(B, H, H * D)
    # extract each head's diagonal block
    out = np.empty((B, H, D), np.float32)
    for h in range(H):
        out[:, h, :] = full[:, h, h * D : (h + 1) * D]
    return out


if __name__ == "__main__":
    # smoke test with random inputs
    rng = np.random.default_rng(0)
    inputs = {
        "q": rng.standard_normal((B, H, D), dtype=np.float32),
        "k": rng.standard_normal((B, H, D), dtype=np.float32),
        "v": rng.standard_normal((B, H, D), dtype=np.float32),
        "k_cache": rng.standard_normal((NBLK, BS, H, D), dtype=np.float32),
        "v_cache": rng.standard_normal((NBLK, BS, H, D), dtype=np.float32),
        "slot_mapping": np.arange(B, dtype=np.int32) * T + (T - 1),
        "block_tables": np.arange(B * BPS, dtype=np.int32).reshape(B, BPS),
        "context_lens": np.full((B,), T, dtype=np.int32),
    }
    print(kernel(**inputs).shape)
